# revision 36
# baseline (speedup 1.0000x reference)
"""Trainium2 Bass kernel for nn_CrossProduct (factorization-machine cross term).

out_b = 0.5 * [ sum_k (x_b @ v_k)^2  -  sum_i w_i x_bi^2 ],  w_i = sum_k v_ik^2

Host-side rescaling removes all per-feature weights from the device:
  x~  = e3m4(4 * x * sqrt(w/2))     (shipped fp8e3m4, feature-on-partition)
  v~  = fp16(v / sqrt(w))           (bit-cast into the head of the X tensor)
  psA[k,b] = sum_i x~_bi v~_ik = 4 * (x v_k)/sqrt(2)     (mixed-dtype matmul)
  sq = (psA/4)^2 fp16, weighted by fp16(CBIAS) in the final reduce
  x2 = e4m3(CBIAS * x~^2) pair-interleaved; reduced on the PE via four
  DoubleRow ones(-1/16) passes (256-deep contraction each).  CBIAS is a
  non-pow2 scale that breaks e3m4-grid alignment so the e4m3 RTN cast of
  squares is mean-unbiased (plain cast has +0.75% convexity bias).
  out_b = (sum_k c*sq - (1/16) sum x2) / c  accumulated in psO row 0.

Schedule highlights:
  - Single dram tensor XR [128, 17412] e3m4, chunk order
    [vw | c0 | c1 | c6 | c7 | c2 | c3 | c4 | c5].  DMAs ride the two
    HWDGE queues (SWDGE completion semaphores lag ~2.2us): sync sends
    [vw,c0], [c1,c6], [c7]; scalar sends [c2,c3], [c4,c5].  3-6KB
    descriptors keep both queues near the 16-DMA-engine aggregate
    (~21ns/packet queue pacing makes small descriptors the bottleneck).
  - 8 PE warm-up matmuls on a memset tile (no DMA dependency) raise the
    p-state clock during the framework preamble; pa then streams at
    ~215ns per 512-col matmul (2.4GHz effective).
  - squares split ACT/DVE full-chunk in arrival order; Pool does no
    tensor ops (they contend with DVE for SBUF ports and run at ~0.4x).
  - finals weights-major (one ldweights per set); the last-ready x2
    pair (4,5) closes each psO bank region so the post-last-square
    critical path is 4 DR instructions; psO-bank-aligned output copies.
"""

import math
from contextlib import ExitStack

import ml_dtypes
import numpy as np

import concourse.bass as bass
import concourse.bacc as bacc
import concourse.mybir as mybir
import concourse.tile as tile
from concourse.bass_utils import run_bass_kernel_spmd

F16 = mybir.dt.float16
F32 = mybir.dt.float32
F8E3 = mybir.dt.float8e3
F8E4 = mybir.dt.float8e4

N_CORES = 8
B, XD, KD = 16384, 1024, 64
BS = B // N_CORES   # 2048 batch rows per core
C = XD // 128       # 8 contraction chunks of 128
ALPHA = 4.0         # x pre-scale (exact pow2); term2 weight = 1/ALPHA^2
CBIAS = 1.17        # x2 pre-cast scale nulling the e4m3 RTN square bias
VWB = 1028          # vw bytes per partition at the head of XR
XRW = VWB + C * BS  # 17412 bytes per partition

# chunk order inside XR (after vw), chosen so the sync queue can deliver
# c0 early and both queues balance.
XORD = [0, 1, 6, 7, 2, 3, 4, 5]  # slots: c0 | c1,c6 | c7 | c2,c3 | c4,c5
XPOS = {c: i for i, c in enumerate(XORD)}  # chunk -> slot

SQ = mybir.ActivationFunctionType.Square
DR = mybir.MatmulPerfMode.DoubleRow


def _body(ctx, tc, OUT, XR):
    nc = tc.nc
    const = ctx.enter_context(tc.tile_pool(name="const", bufs=1))
    xpool = ctx.enter_context(tc.tile_pool(name="xp", bufs=1))
    x2pool = ctx.enter_context(tc.tile_pool(name="x2p", bufs=1))
    sqpool = ctx.enter_context(tc.tile_pool(name="sqp", bufs=1))
    opool = ctx.enter_context(tc.tile_pool(name="op", bufs=1))
    psa = ctx.enter_context(tc.tile_pool(name="psA", bufs=1, space="PSUM"))
    pso = ctx.enter_context(tc.tile_pool(name="psO", bufs=1, space="PSUM"))

    pa = psa.tile([64, BS], F32)
    po = pso.tile([1, BS], F32)

    # ---- PE warm-up (alternating psO bank regions so they pipeline).
    warm = const.tile([128, 512], F16)
    nc.vector.memset(warm, 0.0)
    for i in range(8):
        cols = slice(512 * (i % 2), 512 * (i % 2) + 512)
        nc.tensor.matmul(
            po[0:1, cols], warm[:, 0:1], warm[:],
            start=True, stop=True, tile_position=(0, 0),
        )

    # ---- XR input.
    xr = xpool.tile([128, XRW], F8E3)

    def xsl(lo, hi):  # slot-range bounds (bytes, after vw head)
        return slice(VWB + lo * BS, VWB + hi * BS)

    # Both X streams ride the two HWDGE queues: SWDGE (gpsimd) completion
    # semaphores lag ~2.2us behind the last packet, HWDGE ones don't.
    with tc.high_priority():
        nc.sync.dma_start(xr[:, 0 : VWB + BS], XR[:, 0 : VWB + BS])  # vw,c0
        nc.scalar.dma_start(xr[:, xsl(4, 6)], XR[:, xsl(4, 6)])      # c2,c3
    nc.sync.dma_start(xr[:, xsl(1, 3)], XR[:, xsl(1, 3)])            # c1,c6
    nc.scalar.dma_start(xr[:, xsl(6, 8)], XR[:, xsl(6, 8)])          # c4,c5
    nc.sync.dma_start(xr[:, xsl(3, 4)], XR[:, xsl(3, 4)])            # c7

    # vw view: [128, 514] fp16 = [c*64+k -> v~ ; col 512 = +CBIAS(fp16)]
    vw = xr[:, 0:VWB].bitcast(F16)

    def xch(c):
        i = XPOS[c]
        return xr[:, VWB + i * BS : VWB + (i + 1) * BS]

    # ---- pa matmuls in arrival order: fp16 weights x e3m4 moving.
    def pa_mm(c, first, last):
        for q in range(4):
            nc.tensor.matmul(
                pa[:, q * 512 : (q + 1) * 512],
                vw[:, c * KD : (c + 1) * KD],
                xch(c)[:, q * 512 : (q + 1) * 512],
                start=first,
                stop=last,
                tile_position=(0, 0),
            )

    order = [0, 2, 3, 1, 6, 4, 5, 7]
    for i, c in enumerate(order):
        pa_mm(c, i == 0, i == len(order) - 1)

    # ---- squares: x2 = e4m3(CBIAS * x~^2), pair-interleaved for DR.
    # x2[:, cc, j, :] holds chunk (2cc+j)^2.
    x2 = x2pool.tile([128, C // 2, 2, BS], F8E4)
    H = BS // 2
    CSQ = math.sqrt(CBIAS)

    def x2ap(c, h):
        return x2[:, c // 2, c % 2, h * H : (h + 1) * H]

    def sq_act(c, h):
        nc.scalar.activation(x2ap(c, h), xch(c)[:, h * H : (h + 1) * H], SQ,
                             scale=CSQ)

    def sq_dve(c, h):
        nc.vector.scalar_tensor_tensor(
            x2ap(c, h), xch(c)[:, h * H : (h + 1) * H], CBIAS,
            xch(c)[:, h * H : (h + 1) * H],
            mybir.AluOpType.mult, mybir.AluOpType.mult)

    def sq_act_full(c):
        nc.scalar.activation(x2[:, c // 2, c % 2], xch(c), SQ, scale=CSQ)

    def sq_dve_full(c):
        nc.vector.scalar_tensor_tensor(
            x2[:, c // 2, c % 2], xch(c), CBIAS, xch(c),
            mybir.AluOpType.mult, mybir.AluOpType.mult)

    # arrival: c0 ~10.4us; c2,c3 ~12.4; c1,c6 ~13.4; c4,c5 ~15.2; c7 ~16.1.
    # Full-chunk ops amortize per-op overhead; halves only on the shared
    # first chunk.  sq (psA^2) slots into ACT before chunk 4 so the PE's
    # sq pass isn't gated by ACT's square backlog.
    sqs = sqpool.tile([64, BS], F16)
    sq_act(0, 0)
    for c in (2, 6, 4):
        sq_act_full(c)
    nc.scalar.activation(sqs[:], pa[:], SQ, scale=1.0 / ALPHA)
    sq_dve(0, 1)
    for c in (3, 1, 7, 5):
        sq_dve_full(c)

    # ---- finals into psO row 0: weights-major order (one ldweights per
    # weight set): 4 DR ones(-1/16) passes over x2, then the +c sq pass.
    wneg8 = const.tile([128, 2, 16], F8E4)
    nc.vector.memset(wneg8, -1.0 / (ALPHA * ALPHA))
    onesc = vw[0:64, 512:513]

    def dr_pass(cc, first, last):
        for q in range(4):
            cols = slice(q * 512, (q + 1) * 512)
            nc.tensor.matmul(
                po[0:1, cols],
                wneg8[:, :, 0:1],
                x2[:, cc, :, cols],
                start=first,
                stop=last,
                perf_mode=DR,
                tile_position=(0, 0),
            )

    # x2-pair readiness order: (2,3), (0,1), sq, (6,7); warm-ups bridge the
    # PE idle gap so the clock stays up; the last-ready pair (4,5) closes
    # each region so the post-last-square critical path is 4 DR insts.
    dr_pass(1, True, False)
    dr_pass(0, False, False)
    dr_pass(3, False, False)
    for q in range(4):
        cols = slice(q * 512, (q + 1) * 512)
        nc.tensor.matmul(
            po[0:1, cols],
            onesc,
            sqs[:, cols],
            start=False,
            stop=False,
            tile_position=(0, 0),
        )
    dr_pass(2, False, True)

    # psO-bank-aligned quarters so each copy is gated only by its own
    # region's accumulation stop.
    outs = opool.tile([1, BS], F16)
    nc.scalar.mul(outs[0:1, 0:512], po[0:1, 0:512], 1.0 / CBIAS)
    nc.vector.tensor_scalar_mul(outs[0:1, 512:1024], po[0:1, 512:1024],
                                1.0 / CBIAS)
    nc.scalar.mul(outs[0:1, 1024:1536], po[0:1, 1024:1536], 1.0 / CBIAS)
    nc.vector.tensor_scalar_mul(outs[0:1, 1536:2048], po[0:1, 1536:2048],
                                1.0 / CBIAS)
    nc.sync.dma_start(OUT[:, 0:1024], outs[0:1, 0:1024])
    nc.scalar.dma_start(OUT[:, 1024:2048], outs[0:1, 1024:2048])


_NC_CACHE = None


def build_nc():
    global _NC_CACHE
    if _NC_CACHE is not None:
        return _NC_CACHE
    nc = bacc.Bacc("TRN2", target_bir_lowering=False, debug=False)
    XR = nc.dram_tensor("XR", [128, XRW], F8E3, kind="ExternalInput").ap()
    OUT = nc.dram_tensor("OUT", [1, BS], F16, kind="ExternalOutput").ap()
    with tile.TileContext(nc) as tc:
        with ExitStack() as ctx:
            _body(ctx, tc, OUT, XR)
    nc.compile()
    _NC_CACHE = nc
    return nc


def make_in_maps(x, vparam):
    x = np.ascontiguousarray(x, dtype=np.float32)
    v = np.ascontiguousarray(vparam, dtype=np.float32)

    w = (v.astype(np.float64) ** 2).sum(axis=1)          # (1024,)
    w = np.maximum(w, 1e-12)
    s = np.sqrt(w / 2.0)
    vn = (v / np.sqrt(w)[:, None]).astype(np.float32)    # (1024, 64)

    VWh = np.empty((128, VWB // 2), dtype=np.float16)
    # VW[p, c*64+k] = vn[c*128+p, k]
    VWh[:, 0 : C * KD] = (
        vn.reshape(C, 128, KD).transpose(1, 0, 2).reshape(128, C * KD)
    )
    VWh[:, C * KD] = np.float16(CBIAS)   # +c weights for the sq pass
    VWh[:, C * KD + 1] = 0.0
    vw_bytes = VWh.view(np.uint8)                        # (128, 1028)

    xs_all = (ALPHA * x * s[None, :]).astype(ml_dtypes.float8_e3m4)  # (B, 1024)

    in_maps = []
    for i in range(N_CORES):
        xs = xs_all[i * BS : (i + 1) * BS]               # (2048, 1024)
        xT = xs.T.reshape(C, 128, BS)                    # [c, p, b]
        xb = np.ascontiguousarray(
            xT[XORD].transpose(1, 0, 2).reshape(128, C * BS)
        ).view(np.uint8)
        XRb = np.concatenate([vw_bytes, xb], axis=1)     # (128, 17412)
        in_maps.append({"XR": XRb.view(ml_dtypes.float8_e3m4)})
    return in_maps


LAST_RESULTS = None  # stashed BassKernelResults (for test harness profiling)
TRACE = False


def kernel(x, vparam):
    global LAST_RESULTS
    nc = build_nc()
    in_maps = make_in_maps(x, vparam)
    res = run_bass_kernel_spmd(nc, in_maps, list(range(N_CORES)), trace=TRACE)
    LAST_RESULTS = res
    out = np.concatenate(
        [
            res.results[i]["OUT"].astype(np.float32).reshape(BS, 1)
            for i in range(N_CORES)
        ],
        axis=0,
    )
    return out.astype(np.float32)


# revision 37
# speedup vs baseline: 1.0043x; 1.0043x over previous
"""Trainium2 Bass kernel for nn_CrossProduct (factorization-machine cross term).

out_b = 0.5 * [ sum_k (x_b @ v_k)^2  -  sum_i w_i x_bi^2 ],  w_i = sum_k v_ik^2

Host-side rescaling removes all per-feature weights from the device:
  x~  = e3m4(4 * x * sqrt(w/2))     (shipped fp8e3m4, feature-on-partition)
  v~  = fp16(v / sqrt(w))           (bit-cast into the head of the X tensor)
  psA[k,b] = sum_i x~_bi v~_ik = 4 * (x v_k)/sqrt(2)     (mixed-dtype matmul)
  sq = (psA/4)^2 fp16, weighted by fp16(CBIAS) in the final reduce
  x2 = e4m3(CBIAS * x~^2) pair-interleaved; reduced on the PE via four
  DoubleRow ones(-1/16) passes (256-deep contraction each).  CBIAS is a
  non-pow2 scale that breaks e3m4-grid alignment so the e4m3 RTN cast of
  squares is mean-unbiased (plain cast has +0.75% convexity bias).
  out_b = (sum_k c*sq - (1/16) sum x2) / c  accumulated in psO row 0.

Schedule highlights:
  - Single dram tensor XR [128, 17412] e3m4, chunk order
    [vw | c0 | c1 | c6 | c7 | c2 | c3 | c4 | c5].  DMAs ride the two
    HWDGE queues (SWDGE completion semaphores lag ~2.2us): sync sends
    [vw,c0], [c1,c6], [c7]; scalar sends [c2,c3], [c4,c5].  3-6KB
    descriptors keep both queues near the 16-DMA-engine aggregate
    (~21ns/packet queue pacing makes small descriptors the bottleneck).
  - 8 PE warm-up matmuls on a memset tile (no DMA dependency) raise the
    p-state clock during the framework preamble; pa then streams at
    ~215ns per 512-col matmul (2.4GHz effective).
  - squares split ACT/DVE full-chunk in arrival order; Pool does no
    tensor ops (they contend with DVE for SBUF ports and run at ~0.4x).
  - finals weights-major (one ldweights per set); the last-ready x2
    pair (4,5) closes each psO bank region so the post-last-square
    critical path is 4 DR instructions; psO-bank-aligned output copies.
"""

import math
from contextlib import ExitStack

import ml_dtypes
import numpy as np

import concourse.bass as bass
import concourse.bacc as bacc
import concourse.mybir as mybir
import concourse.tile as tile
from concourse.bass_utils import run_bass_kernel_spmd

F16 = mybir.dt.float16
F32 = mybir.dt.float32
F8E3 = mybir.dt.float8e3
F8E4 = mybir.dt.float8e4

N_CORES = 8
B, XD, KD = 16384, 1024, 64
BS = B // N_CORES   # 2048 batch rows per core
C = XD // 128       # 8 contraction chunks of 128
ALPHA = 4.0         # x pre-scale (exact pow2); term2 weight = 1/ALPHA^2
CBIAS = 1.17        # x2 pre-cast scale nulling the e4m3 RTN square bias
VWB = 1028          # vw bytes per partition at the head of XR
XRW = VWB + C * BS  # 17412 bytes per partition

# chunk order inside XR (after vw), chosen so the sync queue can deliver
# c0 early and both queues balance.
XORD = [0, 1, 6, 7, 2, 3, 4, 5]  # slots: c0 | c1,c6 | c7 | c2,c3 | c4,c5
XPOS = {c: i for i, c in enumerate(XORD)}  # chunk -> slot

SQ = mybir.ActivationFunctionType.Square
DR = mybir.MatmulPerfMode.DoubleRow


def _body(ctx, tc, OUT, XR):
    nc = tc.nc
    const = ctx.enter_context(tc.tile_pool(name="const", bufs=1))
    xpool = ctx.enter_context(tc.tile_pool(name="xp", bufs=1))
    x2pool = ctx.enter_context(tc.tile_pool(name="x2p", bufs=1))
    sqpool = ctx.enter_context(tc.tile_pool(name="sqp", bufs=1))
    opool = ctx.enter_context(tc.tile_pool(name="op", bufs=1))
    psa = ctx.enter_context(tc.tile_pool(name="psA", bufs=1, space="PSUM"))
    pso = ctx.enter_context(tc.tile_pool(name="psO", bufs=1, space="PSUM"))

    pa = psa.tile([64, BS], F32)
    po = pso.tile([1, BS], F32)

    # ---- PE warm-up (alternating psO bank regions so they pipeline).
    warm = const.tile([128, 512], F16)
    nc.vector.memset(warm, 0.0)
    for i in range(8):
        cols = slice(512 * (i % 2), 512 * (i % 2) + 512)
        nc.tensor.matmul(
            po[0:1, cols], warm[:, 0:1], warm[:],
            start=True, stop=True, tile_position=(0, 0),
        )

    # ---- XR input.
    xr = xpool.tile([128, XRW], F8E3)

    def xsl(lo, hi):  # slot-range bounds (bytes, after vw head)
        return slice(VWB + lo * BS, VWB + hi * BS)

    # Both X streams ride the two HWDGE queues: SWDGE (gpsimd) completion
    # semaphores lag ~2.2us behind the last packet, HWDGE ones don't.
    with tc.high_priority():
        nc.sync.dma_start(xr[:, 0 : VWB + BS], XR[:, 0 : VWB + BS])  # vw,c0
        nc.scalar.dma_start(xr[:, xsl(4, 6)], XR[:, xsl(4, 6)])      # c2,c3
    nc.sync.dma_start(xr[:, xsl(1, 3)], XR[:, xsl(1, 3)])            # c1,c6
    nc.scalar.dma_start(xr[:, xsl(6, 8)], XR[:, xsl(6, 8)])          # c4,c5
    nc.sync.dma_start(xr[:, xsl(3, 4)], XR[:, xsl(3, 4)])            # c7

    # vw view: [128, 514] fp16 = [c*64+k -> v~ ; col 512 = +CBIAS(fp16)]
    vw = xr[:, 0:VWB].bitcast(F16)

    def xch(c):
        i = XPOS[c]
        return xr[:, VWB + i * BS : VWB + (i + 1) * BS]

    # ---- pa matmuls in arrival order: fp16 weights x e3m4 moving.
    def pa_mm(c, first, last):
        for q in range(4):
            nc.tensor.matmul(
                pa[:, q * 512 : (q + 1) * 512],
                vw[:, c * KD : (c + 1) * KD],
                xch(c)[:, q * 512 : (q + 1) * 512],
                start=first,
                stop=last,
                tile_position=(0, 0),
            )

    order = [0, 2, 3, 1, 6, 4, 5, 7]
    for i, c in enumerate(order):
        pa_mm(c, i == 0, i == len(order) - 1)

    # ---- squares: x2 = e4m3(CBIAS * x~^2), pair-interleaved for DR.
    # x2[:, cc, j, :] holds chunk (2cc+j)^2.
    x2 = x2pool.tile([128, C // 2, 2, BS], F8E4)
    H = BS // 2
    CSQ = math.sqrt(CBIAS)

    def x2ap(c, h):
        return x2[:, c // 2, c % 2, h * H : (h + 1) * H]

    def sq_act(c, h):
        nc.scalar.activation(x2ap(c, h), xch(c)[:, h * H : (h + 1) * H], SQ,
                             scale=CSQ)

    def sq_dve(c, h):
        nc.vector.scalar_tensor_tensor(
            x2ap(c, h), xch(c)[:, h * H : (h + 1) * H], CBIAS,
            xch(c)[:, h * H : (h + 1) * H],
            mybir.AluOpType.mult, mybir.AluOpType.mult)

    def sq_act_full(c):
        nc.scalar.activation(x2[:, c // 2, c % 2], xch(c), SQ, scale=CSQ)

    def sq_dve_full(c):
        nc.vector.scalar_tensor_tensor(
            x2[:, c // 2, c % 2], xch(c), CBIAS, xch(c),
            mybir.AluOpType.mult, mybir.AluOpType.mult)

    # arrival: c0 ~10.4us; c2,c3 ~12.4; c1,c6 ~13.4; c4,c5 ~15.2; c7 ~16.1.
    # Full-chunk ops amortize per-op overhead; halves only on the shared
    # first chunk.  sq (psA^2) slots into ACT before chunk 4 so the PE's
    # sq pass isn't gated by ACT's square backlog.
    sqs = sqpool.tile([64, BS], F16)
    sq_act(0, 0)
    for c in (2, 6, 4):
        sq_act_full(c)
    nc.scalar.activation(sqs[:], pa[:], SQ, scale=1.0 / ALPHA)
    sq_dve(0, 1)
    for c in (3, 1, 7, 5):
        sq_dve_full(c)

    # ---- finals into psO row 0: weights-major order (one ldweights per
    # weight set): 4 DR ones(-1/16) passes over x2, then the +c sq pass.
    wneg8 = const.tile([128, 2, 16], F8E4)
    nc.vector.memset(wneg8, -1.0 / (ALPHA * ALPHA))
    onesc = vw[0:64, 512:513]

    def dr_pass(cc, first, last):
        for q in range(4):
            cols = slice(q * 512, (q + 1) * 512)
            nc.tensor.matmul(
                po[0:1, cols],
                wneg8[:, :, 0:1],
                x2[:, cc, :, cols],
                start=first,
                stop=last,
                perf_mode=DR,
                tile_position=(0, 0),
            )

    # x2-pair readiness order: (2,3), (0,1), sq, (6,7); warm-ups bridge the
    # PE idle gap so the clock stays up; the last-ready pair (4,5) closes
    # each region so the post-last-square critical path is 4 DR insts.
    dr_pass(1, True, False)
    dr_pass(0, False, False)
    dr_pass(3, False, False)
    for q in range(4):
        cols = slice(q * 512, (q + 1) * 512)
        nc.tensor.matmul(
            po[0:1, cols],
            onesc,
            sqs[:, cols],
            start=False,
            stop=False,
            tile_position=(0, 0),
        )
    dr_pass(2, False, True)

    # psO-bank-aligned quarters so each copy is gated only by its own
    # region's accumulation stop.
    outs = opool.tile([1, BS], F16)
    nc.scalar.mul(outs[0:1, 0:512], po[0:1, 0:512], 1.0 / CBIAS)
    nc.vector.tensor_scalar_mul(outs[0:1, 512:1024], po[0:1, 512:1024],
                                1.0 / CBIAS)
    nc.scalar.mul(outs[0:1, 1024:1536], po[0:1, 1024:1536], 1.0 / CBIAS)
    nc.vector.tensor_scalar_mul(outs[0:1, 1536:2048], po[0:1, 1536:2048],
                                1.0 / CBIAS)
    nc.sync.dma_start(OUT, outs[0:1, :])


_NC_CACHE = None


def build_nc():
    global _NC_CACHE
    if _NC_CACHE is not None:
        return _NC_CACHE
    nc = bacc.Bacc("TRN2", target_bir_lowering=False, debug=False)
    XR = nc.dram_tensor("XR", [128, XRW], F8E3, kind="ExternalInput").ap()
    OUT = nc.dram_tensor("OUT", [1, BS], F16, kind="ExternalOutput").ap()
    with tile.TileContext(nc) as tc:
        with ExitStack() as ctx:
            _body(ctx, tc, OUT, XR)
    nc.compile()
    _NC_CACHE = nc
    return nc


def make_in_maps(x, vparam):
    x = np.ascontiguousarray(x, dtype=np.float32)
    v = np.ascontiguousarray(vparam, dtype=np.float32)

    w = (v.astype(np.float64) ** 2).sum(axis=1)          # (1024,)
    w = np.maximum(w, 1e-12)
    s = np.sqrt(w / 2.0)
    vn = (v / np.sqrt(w)[:, None]).astype(np.float32)    # (1024, 64)

    VWh = np.empty((128, VWB // 2), dtype=np.float16)
    # VW[p, c*64+k] = vn[c*128+p, k]
    VWh[:, 0 : C * KD] = (
        vn.reshape(C, 128, KD).transpose(1, 0, 2).reshape(128, C * KD)
    )
    VWh[:, C * KD] = np.float16(CBIAS)   # +c weights for the sq pass
    VWh[:, C * KD + 1] = 0.0
    vw_bytes = VWh.view(np.uint8)                        # (128, 1028)

    xs_all = (ALPHA * x * s[None, :]).astype(ml_dtypes.float8_e3m4)  # (B, 1024)

    in_maps = []
    for i in range(N_CORES):
        xs = xs_all[i * BS : (i + 1) * BS]               # (2048, 1024)
        xT = xs.T.reshape(C, 128, BS)                    # [c, p, b]
        xb = np.ascontiguousarray(
            xT[XORD].transpose(1, 0, 2).reshape(128, C * BS)
        ).view(np.uint8)
        XRb = np.concatenate([vw_bytes, xb], axis=1)     # (128, 17412)
        in_maps.append({"XR": XRb.view(ml_dtypes.float8_e3m4)})
    return in_maps


LAST_RESULTS = None  # stashed BassKernelResults (for test harness profiling)
TRACE = False


def kernel(x, vparam):
    global LAST_RESULTS
    nc = build_nc()
    in_maps = make_in_maps(x, vparam)
    res = run_bass_kernel_spmd(nc, in_maps, list(range(N_CORES)), trace=TRACE)
    LAST_RESULTS = res
    out = np.concatenate(
        [
            res.results[i]["OUT"].astype(np.float32).reshape(BS, 1)
            for i in range(N_CORES)
        ],
        axis=0,
    )
    return out.astype(np.float32)


# revision 38
# speedup vs baseline: 1.0130x; 1.0086x over previous
"""Trainium2 Bass kernel for nn_CrossProduct (factorization-machine cross term).

out_b = 0.5 * [ sum_k (x_b @ v_k)^2  -  sum_i w_i x_bi^2 ],  w_i = sum_k v_ik^2

Host-side rescaling removes all per-feature weights from the device:
  x~  = e3m4(4 * x * sqrt(w/2))     (shipped fp8e3m4, feature-on-partition)
  v~  = fp16(v / sqrt(w))           (bit-cast into the head of the X tensor)
  psA[k,b] = sum_i x~_bi v~_ik = 4 * (x v_k)/sqrt(2)     (mixed-dtype matmul)
  sq = (psA/4)^2 fp16, weighted by fp16(CBIAS) in the final reduce
  x2 = e4m3(CBIAS * x~^2) pair-interleaved; reduced on the PE via four
  DoubleRow ones(-1/16) passes (256-deep contraction each).  CBIAS is a
  non-pow2 scale that breaks e3m4-grid alignment so the e4m3 RTN cast of
  squares is mean-unbiased (plain cast has +0.75% convexity bias).
  out_b = (sum_k c*sq - (1/16) sum x2) / c  accumulated in psO row 0.

Schedule highlights:
  - Single dram tensor XR [128, 17412] e3m4, chunk order
    [vw | c0 | c1 | c6 | c7 | c2 | c3 | c4 | c5].  DMAs ride the two
    HWDGE queues (SWDGE completion semaphores lag ~2.2us): sync sends
    [vw,c0], [c1,c6], [c7]; scalar sends [c2,c3], [c4,c5].  3-6KB
    descriptors keep both queues near the 16-DMA-engine aggregate
    (~21ns/packet queue pacing makes small descriptors the bottleneck).
  - 8 PE warm-up matmuls on a memset tile (no DMA dependency) raise the
    p-state clock during the framework preamble; pa then streams at
    ~215ns per 512-col matmul (2.4GHz effective).
  - squares split ACT/DVE full-chunk in arrival order; Pool does no
    tensor ops (they contend with DVE for SBUF ports and run at ~0.4x).
  - finals weights-major (one ldweights per set); the last-ready x2
    pair (4,5) closes each psO bank region so the post-last-square
    critical path is 4 DR instructions; psO-bank-aligned output copies.
"""

import math
from contextlib import ExitStack

import ml_dtypes
import numpy as np

import concourse.bass as bass
import concourse.bacc as bacc
import concourse.mybir as mybir
import concourse.tile as tile
from concourse.bass_utils import run_bass_kernel_spmd

F16 = mybir.dt.float16
F32 = mybir.dt.float32
F8E3 = mybir.dt.float8e3
F8E4 = mybir.dt.float8e4

N_CORES = 8
B, XD, KD = 16384, 1024, 64
BS = B // N_CORES   # 2048 batch rows per core
C = XD // 128       # 8 contraction chunks of 128
ALPHA = 4.0         # x pre-scale (exact pow2); term2 weight = 1/ALPHA^2
CBIAS = 1.17        # x2 pre-cast scale nulling the e4m3 RTN square bias
VWB = 1028          # vw bytes per partition at the head of XR
XRW = VWB + C * BS  # 17412 bytes per partition

# chunk order inside XR (after vw), chosen so the sync queue can deliver
# c0 early and both queues balance.
XORD = [0, 1, 6, 7, 2, 3, 4, 5]  # slots: c0 | c1,c6 | c7 | c2,c3 | c4,c5
XPOS = {c: i for i, c in enumerate(XORD)}  # chunk -> slot

SQ = mybir.ActivationFunctionType.Square
DR = mybir.MatmulPerfMode.DoubleRow


def _body(ctx, tc, OUT, XR):
    nc = tc.nc
    const = ctx.enter_context(tc.tile_pool(name="const", bufs=1))
    xpool = ctx.enter_context(tc.tile_pool(name="xp", bufs=1))
    x2pool = ctx.enter_context(tc.tile_pool(name="x2p", bufs=1))
    sqpool = ctx.enter_context(tc.tile_pool(name="sqp", bufs=1))
    opool = ctx.enter_context(tc.tile_pool(name="op", bufs=1))
    psa = ctx.enter_context(tc.tile_pool(name="psA", bufs=1, space="PSUM"))
    pso = ctx.enter_context(tc.tile_pool(name="psO", bufs=1, space="PSUM"))

    pa = psa.tile([64, BS], F32)
    po = pso.tile([1, BS], F32)

    # ---- PE warm-up (alternating psO bank regions so they pipeline).
    warm = const.tile([128, 512], F16)
    nc.vector.memset(warm, 0.0)
    for i in range(8):
        cols = slice(512 * (i % 2), 512 * (i % 2) + 512)
        nc.tensor.matmul(
            po[0:1, cols], warm[:, 0:1], warm[:],
            start=True, stop=True, tile_position=(0, 0),
        )

    # ---- XR input.
    xr = xpool.tile([128, XRW], F8E3)

    def xsl(lo, hi):  # slot-range bounds (bytes, after vw head)
        return slice(VWB + lo * BS, VWB + hi * BS)

    # Both X streams ride the two HWDGE queues: SWDGE (gpsimd) completion
    # semaphores lag ~2.2us behind the last packet, HWDGE ones don't.
    with tc.high_priority():
        nc.sync.dma_start(xr[:, 0 : VWB + BS], XR[:, 0 : VWB + BS])  # vw,c0
        nc.scalar.dma_start(xr[:, xsl(4, 6)], XR[:, xsl(4, 6)])      # c2,c3
    nc.sync.dma_start(xr[:, xsl(1, 3)], XR[:, xsl(1, 3)])            # c1,c6
    nc.scalar.dma_start(xr[:, xsl(6, 8)], XR[:, xsl(6, 8)])          # c4,c5
    nc.sync.dma_start(xr[:, xsl(3, 4)], XR[:, xsl(3, 4)])            # c7

    # vw view: [128, 514] fp16 = [c*64+k -> v~ ; col 512 = +CBIAS(fp16)]
    vw = xr[:, 0:VWB].bitcast(F16)

    def xch(c):
        i = XPOS[c]
        return xr[:, VWB + i * BS : VWB + (i + 1) * BS]

    # ---- pa matmuls in arrival order: fp16 weights x e3m4 moving.
    def pa_mm(c, first, last):
        for q in range(4):
            nc.tensor.matmul(
                pa[:, q * 512 : (q + 1) * 512],
                vw[:, c * KD : (c + 1) * KD],
                xch(c)[:, q * 512 : (q + 1) * 512],
                start=first,
                stop=last,
                tile_position=(0, 0),
            )

    order = [0, 2, 3, 1, 6, 4, 5, 7]
    for i, c in enumerate(order):
        pa_mm(c, i == 0, i == len(order) - 1)

    # ---- squares: x2 = e4m3(CBIAS * x~^2), pair-interleaved for DR.
    # x2[:, cc, j, :] holds chunk (2cc+j)^2.
    x2 = x2pool.tile([128, C // 2, 2, BS], F8E4)
    H = BS // 2
    CSQ = math.sqrt(CBIAS)

    def x2ap(c, h):
        return x2[:, c // 2, c % 2, h * H : (h + 1) * H]

    def sq_act(c, h):
        nc.scalar.activation(x2ap(c, h), xch(c)[:, h * H : (h + 1) * H], SQ,
                             scale=CSQ)

    def sq_dve(c, h):
        nc.vector.scalar_tensor_tensor(
            x2ap(c, h), xch(c)[:, h * H : (h + 1) * H], CBIAS,
            xch(c)[:, h * H : (h + 1) * H],
            mybir.AluOpType.mult, mybir.AluOpType.mult)

    def sq_act_full(c):
        nc.scalar.activation(x2[:, c // 2, c % 2], xch(c), SQ, scale=CSQ)

    def sq_dve_full(c):
        nc.vector.scalar_tensor_tensor(
            x2[:, c // 2, c % 2], xch(c), CBIAS, xch(c),
            mybir.AluOpType.mult, mybir.AluOpType.mult)

    # arrival: c0 ~10.4us; c2,c3 ~12.4; c1,c6 ~13.4; c4,c5 ~15.2; c7 ~16.1.
    # Full-chunk ops amortize per-op overhead; halves only on the shared
    # first chunk.  sq (psA^2) slots into ACT before chunk 4 so the PE's
    # sq pass isn't gated by ACT's square backlog.
    sqs = sqpool.tile([64, BS], F16)
    sq_act(0, 0)
    for c in (2, 6, 4):
        sq_act_full(c)
    nc.scalar.activation(sqs[:, 0:H], pa[:, 0:H], SQ, scale=1.0 / ALPHA)
    nc.scalar.activation(sqs[:, H:BS], pa[:, H:BS], SQ, scale=1.0 / ALPHA)
    sq_dve(0, 1)
    for c in (3, 1, 7, 5):
        sq_dve_full(c)

    # ---- finals into psO row 0: weights-major order (one ldweights per
    # weight set): 4 DR ones(-1/16) passes over x2, then the +c sq pass.
    wneg8 = const.tile([128, 2, 16], F8E4)
    nc.vector.memset(wneg8, -1.0 / (ALPHA * ALPHA))
    onesc = vw[0:64, 512:513]

    def dr_pass(cc, first, last):
        for q in range(4):
            cols = slice(q * 512, (q + 1) * 512)
            nc.tensor.matmul(
                po[0:1, cols],
                wneg8[:, :, 0:1],
                x2[:, cc, :, cols],
                start=first,
                stop=last,
                perf_mode=DR,
                tile_position=(0, 0),
            )

    # x2-pair readiness order: (2,3), (0,1), sq, (6,7); warm-ups bridge the
    # PE idle gap so the clock stays up; the last-ready pair (4,5) closes
    # each region so the post-last-square critical path is 4 DR insts.
    dr_pass(1, True, False)
    dr_pass(0, False, False)
    dr_pass(3, False, False)
    for q in range(4):
        cols = slice(q * 512, (q + 1) * 512)
        nc.tensor.matmul(
            po[0:1, cols],
            onesc,
            sqs[:, cols],
            start=False,
            stop=False,
            tile_position=(0, 0),
        )
    dr_pass(2, False, True)

    # psO-bank-aligned quarters so each copy is gated only by its own
    # region's accumulation stop.
    outs = opool.tile([1, BS], F16)
    nc.scalar.mul(outs[0:1, 0:512], po[0:1, 0:512], 1.0 / CBIAS)
    nc.vector.tensor_scalar_mul(outs[0:1, 512:1024], po[0:1, 512:1024],
                                1.0 / CBIAS)
    nc.scalar.mul(outs[0:1, 1024:1536], po[0:1, 1024:1536], 1.0 / CBIAS)
    nc.vector.tensor_scalar_mul(outs[0:1, 1536:2048], po[0:1, 1536:2048],
                                1.0 / CBIAS)
    nc.sync.dma_start(OUT, outs[0:1, :])


_NC_CACHE = None


def build_nc():
    global _NC_CACHE
    if _NC_CACHE is not None:
        return _NC_CACHE
    nc = bacc.Bacc("TRN2", target_bir_lowering=False, debug=False)
    XR = nc.dram_tensor("XR", [128, XRW], F8E3, kind="ExternalInput").ap()
    OUT = nc.dram_tensor("OUT", [1, BS], F16, kind="ExternalOutput").ap()
    with tile.TileContext(nc) as tc:
        with ExitStack() as ctx:
            _body(ctx, tc, OUT, XR)
    nc.compile()
    _NC_CACHE = nc
    return nc


def make_in_maps(x, vparam):
    x = np.ascontiguousarray(x, dtype=np.float32)
    v = np.ascontiguousarray(vparam, dtype=np.float32)

    w = (v.astype(np.float64) ** 2).sum(axis=1)          # (1024,)
    w = np.maximum(w, 1e-12)
    s = np.sqrt(w / 2.0)
    vn = (v / np.sqrt(w)[:, None]).astype(np.float32)    # (1024, 64)

    VWh = np.empty((128, VWB // 2), dtype=np.float16)
    # VW[p, c*64+k] = vn[c*128+p, k]
    VWh[:, 0 : C * KD] = (
        vn.reshape(C, 128, KD).transpose(1, 0, 2).reshape(128, C * KD)
    )
    VWh[:, C * KD] = np.float16(CBIAS)   # +c weights for the sq pass
    VWh[:, C * KD + 1] = 0.0
    vw_bytes = VWh.view(np.uint8)                        # (128, 1028)

    xs_all = (ALPHA * x * s[None, :]).astype(ml_dtypes.float8_e3m4)  # (B, 1024)

    in_maps = []
    for i in range(N_CORES):
        xs = xs_all[i * BS : (i + 1) * BS]               # (2048, 1024)
        xT = xs.T.reshape(C, 128, BS)                    # [c, p, b]
        xb = np.ascontiguousarray(
            xT[XORD].transpose(1, 0, 2).reshape(128, C * BS)
        ).view(np.uint8)
        XRb = np.concatenate([vw_bytes, xb], axis=1)     # (128, 17412)
        in_maps.append({"XR": XRb.view(ml_dtypes.float8_e3m4)})
    return in_maps


LAST_RESULTS = None  # stashed BassKernelResults (for test harness profiling)
TRACE = False


def kernel(x, vparam):
    global LAST_RESULTS
    nc = build_nc()
    in_maps = make_in_maps(x, vparam)
    res = run_bass_kernel_spmd(nc, in_maps, list(range(N_CORES)), trace=TRACE)
    LAST_RESULTS = res
    out = np.concatenate(
        [
            res.results[i]["OUT"].astype(np.float32).reshape(BS, 1)
            for i in range(N_CORES)
        ],
        axis=0,
    )
    return out.astype(np.float32)
